# revision 28
# baseline (speedup 1.0000x reference)
"""Multi-head attention (B=2, N=4096, C=768, H=12, RoPE) on 8 trn2 NeuronCores.

Sharding: (batch, head)-parallel. Core c owns batch b = c//4 and the 3 heads
h in [(c%4)*3, (c%4)*3+3). Each core computes the qkv projection for its
heads, RoPE, full softmax attention, and its partial output projection; the
host sums the 4 partial projections per batch.

v2 pipeline (vs the 891us baseline):
  - Softmax exp split across ScalarE (exact exp LUT) and VectorE (Schraudolph
    int16->bf16 bit-trick fast exp, ~3% per-element; a matched multiplicative
    bias on the exact chunks keeps the softmax mean-consistent). Split ratio
    and fast-exp constants are runtime inputs (no recompile to retune).
  - PV runs at full PE rate: the exp'd score tile ec [128k, 128q] is the
    stationary operand, rhs = V augmented with a ones column [128, 65]; the
    output lands [q, d+z] with the softmax denominator z on the same
    partition as its outputs (per-partition normalize, no broadcast matmul).
    All 4 q-tile accumulators of a 512-q block pack into ONE psum bank
    (start=True clears the whole 2KB zero region; only the first matmul in
    the bank starts, everything else accumulates/overwrites per has_written).
  - The [q, d] output is transposed back to [d, q] staging via PE-transpose.
  - One shared RoPE table (attention scale folded into the q weights/biases);
    RoPE is 2 DVE ops per [128, 512] block: s1 = (qkv_psum + b) * tab, then
    dest[0:64] = s1[0:64] + s1[64:128].
  - Heads are software-pipelined: head h+1's qkv projection + RoPE interleave
    with head h's attention; the out-projection of q-block bp follows head
    2's bp drain. PE never idles long enough to re-throttle (HAM).
"""

import os
import sys

sys.path.insert(0, "/opt/trn_rl_repo")

import numpy as np
import ml_dtypes

B, N, C = 2, 4096, 768
H = 12
HD = 64
HH = HD // 2  # 32
THETA = 10000.0
NCORES = 8
HPC = 3  # heads per core
NT = N // 128  # 32 n-tiles
NBLK = N // 512  # 8 q/v blocks
KT = N // 128  # 32 k-tiles
NCH = KT // 2  # 16 kt-pair chunks per q-block

BF16 = ml_dtypes.bfloat16

# exp split pattern: which chunks go to the DVE fast-exp
KPAT = os.environ.get("KPAT", "kp:2,5,8,12")
if KPAT.startswith("kp:"):
    _KPSET = frozenset(int(v) for v in KPAT[3:].split(","))
    def dve_chunk(g, kp):
        return kp in _KPSET
else:
    _M, _S = (int(v) for v in KPAT.split(":"))
    def dve_chunk(g, kp):
        return g % _M >= _S
PVLAG = int(os.environ.get("KPVLAG", "3"))
# Schraudolph constants (int16 bf16-bits exp): bits = s*EA + EB, with the
# exact-exp chunks biased by CORR = ln(1+mu) to match the fast-exp mean bias.
EA = 128.0 / np.log(2.0)
EB = float(os.environ.get("KEXP_B", str(16256.0 - 5.5)))
CORR = float(os.environ.get("KEXP_CORR", str(np.log1p(0.0102))))

_BUILT = {}


def _rope_tabs():
    inv = 1.0 / (THETA ** (np.arange(0, HD, 2, dtype=np.float64) / HD))  # [32]
    freqs = np.arange(N, dtype=np.float64)[:, None] * inv[None, :]  # [N, 32]
    cos = np.concatenate([np.cos(freqs), np.cos(freqs)], axis=-1)  # [N, 64]
    sin = np.concatenate([np.sin(freqs), np.sin(freqs)], axis=-1)
    cosT = cos.T.astype(np.float32)  # [64, N]
    sinT = sin.T.astype(np.float32)
    # qR[d] = q[d]cos[d] + sgn(d) q[p(d)] sin[d], p(d) = (d+32)%64,
    # sgn = -1 for d<32. s2[r] = (q[r]+b) t2[r]; fold picks row p(d):
    # t2[r] = sgn(p(r)) sin[p(r)] = [sin(freq r) for r<32; -sin(freq r-32)]
    t2 = np.concatenate([sinT[HH:], -sinT[:HH]], axis=0)  # [64, N]
    tabc = np.concatenate([cosT, cosT], axis=0)  # [128, N] for [q; k] rows
    tab2 = np.concatenate([t2, t2], axis=0)  # [128, N]
    return (np.ascontiguousarray(tabc), np.ascontiguousarray(tab2))


def _host_inputs(x, w_qkv, b_qkv, w_proj, b_proj):
    x = np.asarray(x, dtype=np.float32)
    w_qkv = np.asarray(w_qkv, dtype=np.float32)
    b_qkv = np.asarray(b_qkv, dtype=np.float32)
    w_proj = np.asarray(w_proj, dtype=np.float32)
    b_proj = np.asarray(b_proj, dtype=np.float32)

    tabc, tab2 = _rope_tabs()
    scale = HD ** -0.5

    perm = np.concatenate([np.arange(HH, HD), np.arange(0, HH)])  # rotate_half
    wT = w_qkv.T  # [C, 3C]
    wpT = w_proj.T  # [C, C]

    ident = np.eye(128, dtype=BF16)
    foldp = np.zeros((128, 128), dtype=BF16)
    for dd in range(64):
        pd = (dd + 32) % 64
        foldp[pd, dd] = 1            # q rows
        foldp[64 + pd, 64 + dd] = 1  # k rows
    ea = np.full((128, 1), EA, dtype=np.float32)
    eb = np.full((128, 1), EB, dtype=np.float32)
    corr = np.full((128, 1), CORR, dtype=np.float32)

    in_maps = []
    for core in range(NCORES):
        b = core // 4
        h0 = (core % 4) * HPC
        xT = np.ascontiguousarray(x[b].T).astype(BF16)  # [C, N]

        # wqkT: per head ONE e-tile of 128: [q(64, scaled); k(64)]; the
        # rotate-half permutation lives in the fold matrices instead.
        etiles = []
        for h in range(h0, h0 + HPC):
            wq = wT[:, h * HD:(h + 1) * HD] * scale  # [C, 64]
            wk = wT[:, C + h * HD: C + (h + 1) * HD]
            etiles.append(np.concatenate([wq, wk], axis=1))
        wqkT = np.ascontiguousarray(np.concatenate(etiles, axis=1)).astype(BF16)

        wv = np.concatenate(
            [wT[:, 2 * C + h * HD: 2 * C + (h + 1) * HD] for h in range(h0, h0 + HPC)],
            axis=1,
        )  # [768, 192]
        wvT = np.zeros((C, 256), dtype=BF16)
        wvT[:, :192] = wv.astype(BF16)

        bqk = np.zeros((128, HPC), dtype=np.float32)
        for j, h in enumerate(range(h0, h0 + HPC)):
            bqk[:HD, j] = b_qkv[h * HD:(h + 1) * HD] * scale
            bqk[HD:, j] = b_qkv[C + h * HD: C + (h + 1) * HD]
        bv = np.zeros((128, 256), dtype=np.float32)
        for j, h in enumerate(range(h0, h0 + HPC)):
            bv[:, j * HD:(j + 1) * HD] = b_qkv[2 * C + h * HD: 2 * C + (h + 1) * HD]

        # proj weights: heads A,B stacked; head C + bias ones-row
        hA, hB, hC = h0, h0 + 1, h0 + 2
        wp_ab = np.concatenate(
            [wpT[hA * HD:(hA + 1) * HD], wpT[hB * HD:(hB + 1) * HD]], axis=0
        ).astype(BF16)  # [128, 768]
        wp_c = np.zeros((128, C), dtype=np.float32)
        wp_c[:HD] = wpT[hC * HD:(hC + 1) * HD]
        if core % 4 == 0:
            wp_c[HD] = b_proj  # bias once per batch (summed over 4 cores)
        wp_c = wp_c.astype(BF16)

        in_maps.append(
            {
                "xT": xT,
                "wqkT": wqkT,
                "wvT": wvT,
                "bqk": bqk,
                "bv": bv,
                "wp_ab": np.ascontiguousarray(wp_ab),
                "wp_c": np.ascontiguousarray(wp_c),
                "tabc": tabc,
                "tab2": tab2,
                "ident": ident,
                "foldp": foldp,
                "ea": ea,
                "eb": eb,
                "corr": corr,
            }
        )
    return in_maps


def _build_nc():
    import concourse.bass as bass  # noqa: F401
    import concourse.bacc as bacc
    import concourse.tile as tile
    import concourse.mybir as mybir

    f32 = mybir.dt.float32
    bf16 = mybir.dt.bfloat16

    nc = bacc.Bacc("TRN2", num_devices=NCORES, debug=False)

    T = {}
    T["xT_d"] = nc.dram_tensor("xT", [C, N], bf16, kind="ExternalInput").ap()
    T["wqkT_d"] = nc.dram_tensor("wqkT", [C, 384], bf16, kind="ExternalInput").ap()
    T["wvT_d"] = nc.dram_tensor("wvT", [C, 256], bf16, kind="ExternalInput").ap()
    T["bqk_d"] = nc.dram_tensor("bqk", [128, HPC], f32, kind="ExternalInput").ap()
    T["bv_d"] = nc.dram_tensor("bv", [128, 256], f32, kind="ExternalInput").ap()
    T["wpab_d"] = nc.dram_tensor("wp_ab", [128, C], bf16, kind="ExternalInput").ap()
    T["wpc_d"] = nc.dram_tensor("wp_c", [128, C], bf16, kind="ExternalInput").ap()
    T["tabc_d"] = nc.dram_tensor("tabc", [128, N], f32, kind="ExternalInput").ap()
    T["tab2_d"] = nc.dram_tensor("tab2", [128, N], f32, kind="ExternalInput").ap()
    T["ident_d"] = nc.dram_tensor("ident", [128, 128], bf16, kind="ExternalInput").ap()
    T["foldp_d"] = nc.dram_tensor("foldp", [128, 128], bf16, kind="ExternalInput").ap()
    T["ea_d"] = nc.dram_tensor("ea", [128, 1], f32, kind="ExternalInput").ap()
    T["eb_d"] = nc.dram_tensor("eb", [128, 1], f32, kind="ExternalInput").ap()
    T["corr_d"] = nc.dram_tensor("corr", [128, 1], f32, kind="ExternalInput").ap()
    T["out_d"] = nc.dram_tensor("out", [N, C], f32, kind="ExternalOutput").ap()
    taps = {}
    if os.environ.get("KTAPS") == "1":
        for name, shape, dt in (
            ("qt", [128, N], bf16),
            ("kt", [128, N], bf16),
            ("v", [128, NT * HPC * (HD + 1)], bf16),
            ("ec_s", [128, 1024], bf16),
            ("ec_d", [128, 1024], bf16),
            ("sc_d", [128, 1024], f32),
            ("acc", [128, 512], f32),
            ("sp", [128, N], bf16),
            ("scg", [128, N], bf16),
        ):
            taps[name] = nc.dram_tensor(
                "tap_" + name, shape, dt, kind="ExternalOutput"
            ).ap()
    T["taps"] = taps

    with tile.TileContext(nc) as tc:
        _emit(tc, nc, mybir, T)
    nc.compile()
    return nc


def _emit(tc, nc, mybir, T):
    taps = T["taps"]
    f32 = mybir.dt.float32
    f32r = mybir.dt.float32r
    bf16 = mybir.dt.bfloat16
    i16 = mybir.dt.int16
    ALU = mybir.AluOpType
    EXP = mybir.ActivationFunctionType.Exp

    from contextlib import ExitStack

    ctx = ExitStack()
    with ctx:
        const = ctx.enter_context(tc.tile_pool(name="const", bufs=1))
        hw = ctx.enter_context(tc.tile_pool(name="hw", bufs=2))
        qkbuf = ctx.enter_context(tc.tile_pool(name="qkbuf", bufs=2))
        ropes = ctx.enter_context(tc.tile_pool(name="ropes", bufs=3))
        epool = ctx.enter_context(tc.tile_pool(name="epool", bufs=4))
        norms = ctx.enter_context(tc.tile_pool(name="norms", bufs=3))
        fout = ctx.enter_context(tc.tile_pool(name="fout", bufs=3))
        tapp = ctx.enter_context(tc.tile_pool(name="tapp", bufs=1))
        # PSUM: sc 2x[128,1024](4 banks) + acc 2x[128,260](2, shared w/ vp)
        #     + qp 1x[128,512](1, shared w/ transposes) + fp 1x[128,512](1)
        scp = ctx.enter_context(tc.tile_pool(name="scp", bufs=2, space="PSUM"))
        accp = ctx.enter_context(tc.tile_pool(name="accp", bufs=2, space="PSUM"))
        qpp = ctx.enter_context(tc.tile_pool(name="qpp", bufs=1, space="PSUM"))
        fpp = ctx.enter_context(tc.tile_pool(name="fpp", bufs=1, space="PSUM"))

        # ---- constants ----
        xT = const.tile([128, 6, N], bf16, tag="xT")
        for ct in range(6):
            nc.sync.dma_start(out=xT[:, ct, :], in_=T["xT_d"][ct * 128:(ct + 1) * 128, :])
        tabc = const.tile([128, N], f32, tag="tabc")
        nc.sync.dma_start(out=tabc, in_=T["tabc_d"])
        tab2 = const.tile([128, N], f32, tag="tab2")
        nc.sync.dma_start(out=tab2, in_=T["tab2_d"])
        wvT = const.tile([128, 6, 256], bf16, tag="wvT")
        nc.sync.dma_start(out=wvT, in_=T["wvT_d"].rearrange("(t p) e -> p t e", p=128))
        bv = const.tile([128, 256], f32, tag="bv")
        nc.sync.dma_start(out=bv, in_=T["bv_d"])
        bqk = const.tile([128, HPC], f32, tag="bqk")
        nc.sync.dma_start(out=bqk, in_=T["bqk_d"])
        wp_ab = const.tile([128, C], bf16, tag="wp_ab")
        nc.sync.dma_start(out=wp_ab, in_=T["wpab_d"])
        wp_c = const.tile([128, C], bf16, tag="wp_c")
        nc.sync.dma_start(out=wp_c, in_=T["wpc_d"])
        ident = const.tile([128, 128], bf16, tag="ident")
        nc.sync.dma_start(out=ident, in_=T["ident_d"])
        foldp = const.tile([128, 128], bf16, tag="foldp")
        nc.sync.dma_start(out=foldp, in_=T["foldp_d"])
        ea = const.tile([128, 1], f32, tag="ea")
        nc.sync.dma_start(out=ea, in_=T["ea_d"])
        eb = const.tile([128, 1], f32, tag="eb")
        nc.sync.dma_start(out=eb, in_=T["eb_d"])
        corr = const.tile([128, 1], f32, tag="corr")
        nc.sync.dma_start(out=corr, in_=T["corr_d"])

        V_all = const.tile([128, NT, HPC, HD + 1], bf16, tag="V_all")
        for j in range(HPC):
            nc.vector.memset(V_all[:, :, j, HD:HD + 1], 1.0)
        ones_f = const.tile([1, HD], f32, tag="ones_f")
        nc.vector.memset(ones_f, 1.0)
        stagingP = const.tile([128, N], bf16, tag="stagingP")
        stagingC = const.tile([128, N], bf16, tag="stagingC")
        nc.vector.memset(stagingC[HD:HD + 1, :], 1.0)  # proj-bias ones row

        def emit_vproj(nt):
            vp = accp.tile([128, 256], f32, tag="acc", name="vp")
            for ct in range(6):
                nc.tensor.matmul(
                    vp,
                    lhsT=xT[:, ct, nt * 128:(nt + 1) * 128],
                    rhs=wvT[:, ct, :],
                    start=(ct == 0),
                    stop=(ct == 5),
                )
            for j in range(HPC):
                nc.vector.scalar_tensor_tensor(
                    out=V_all[:, nt, j, 0:HD],
                    in0=vp[:, j * HD:(j + 1) * HD],
                    scalar=1.0,
                    in1=bv[:, j * HD:(j + 1) * HD],
                    op0=ALU.mult,
                    op1=ALU.add,
                )

        for _rep in range(int(os.environ.get("KREPEAT", "1"))):

            # per-head state built one head ahead of the attention loop
            wqk_t = [None] * HPC
            QT_t = [None] * HPC
            KT_t = [None] * HPC

            def emit_wqk_dma(h):
                wqk_t[h] = hw.tile([128, 6, 128], bf16, tag="wqk", name=f"wqk{h}")
                nc.sync.dma_start(
                    out=wqk_t[h],
                    in_=T["wqkT_d"].rearrange("(t p) e -> p t e", p=128)[
                        :, :, h * 128:(h + 1) * 128
                    ],
                )
                QT_t[h] = qkbuf.tile([128, N], bf16, tag="QT", name=f"QT{h}")
                KT_t[h] = qkbuf.tile([128, N], bf16, tag="KT", name=f"KT{h}")

            def emit_qkproj_stages(h, blk):
                ns = slice(blk * 512, (blk + 1) * 512)
                st = {}

                def sA1():
                    qp = qpp.tile([128, 512], f32, tag="qp", name="qp")
                    for ct in range(3):
                        nc.tensor.matmul(
                            qp, lhsT=wqk_t[h][:, ct, :], rhs=xT[:, ct, ns],
                            start=(ct == 0), stop=False,
                        )
                    st["qp"] = qp

                def sA2():
                    for ct in range(3, 6):
                        nc.tensor.matmul(
                            st["qp"], lhsT=wqk_t[h][:, ct, :], rhs=xT[:, ct, ns],
                            start=False, stop=(ct == 5),
                        )

                def sB():
                    s1 = ropes.tile([128, 512], bf16, tag="s1", name="s1")
                    nc.vector.scalar_tensor_tensor(
                        out=s1, in0=st["qp"], scalar=bqk[:, h:h + 1],
                        in1=tabc[:, ns], op0=ALU.add, op1=ALU.mult,
                    )
                    s2 = ropes.tile([128, 512], bf16, tag="s2", name="s2")
                    nc.vector.scalar_tensor_tensor(
                        out=s2, in0=st["qp"], scalar=bqk[:, h:h + 1],
                        in1=tab2[:, ns], op0=ALU.add, op1=ALU.mult,
                    )
                    st["s1"], st["s2"] = s1, s2

                def sC():
                    qf = qpp.tile([128, 512], f32, tag="qp", name="qf")
                    nc.tensor.matmul(qf, lhsT=ident, rhs=st["s1"],
                                     start=True, stop=False)
                    nc.tensor.matmul(qf, lhsT=foldp, rhs=st["s2"],
                                     start=False, stop=True)
                    nc.vector.tensor_copy(QT_t[h][0:HD, ns], qf[0:HD, :])
                    nc.vector.tensor_copy(QT_t[h][HD:128, ns], qf[0:HD, :])
                    nc.vector.tensor_copy(KT_t[h][0:HD, ns], qf[HD:128, :])
                    nc.vector.tensor_copy(KT_t[h][HD:128, ns], qf[HD:128, :])

                return [sA1, sA2, sB, sC]

            def emit_qk_dup(h):
                if h == 0 and "qt" in taps:
                    nc.sync.dma_start(out=taps["qt"], in_=QT_t[h])
                    nc.sync.dma_start(out=taps["kt"], in_=KT_t[h])

            # ---- software-pipelined emission ----
            # PE work that depends on DVE/ACT results is deferred so the PE
            # FIFO never waits: PV lags scores/exp by one chunk; drains,
            # next-head qkproj stages and the out-projection are injected
            # one task per chunk from FIFO queues.
            from collections import deque

            emit_wqk_dma(0)
            for stg in emit_qkproj_stages(0, 0):
                stg()
            for nt in range(6):
                emit_vproj(nt)
            emit_wqk_dma(1)

            gchunk = [0]
            tasks = deque()   # drains + out-proj (priority)
            qtasks = deque()  # next-head qkproj stages
            qpopped = [0]
            deferred = [None]  # drain push delayed past the lagging PVs

            def mk_pv(acc, h, ec, kp):
                def pv():
                    for half, kt in ((0, 2 * kp), (1, 2 * kp + 1)):
                        nc.tensor.matmul(
                            acc,
                            lhsT=V_all[:, kt, h, :],
                            rhs=ec[:, half * 512:(half + 1) * 512],
                            start=(kp == 0 and half == 0),
                            stop=(kp == NCH - 1 and half == 1),
                        )
                return pv

            def push_drain(acc, h, bp):
                q0 = slice(bp * 512, (bp + 1) * 512)
                st = {}
                def d1():
                    if h == 0 and bp == 0 and "acc" in taps:
                        at = tapp.tile([128, 512], f32, tag="acct", name="at")
                        nc.vector.tensor_copy(at[0:HD + 1, :], acc)
                        nc.sync.dma_start(out=taps["acc"], in_=at)
                    zc = norms.tile([1, 512], f32, tag="zc", name="zc")
                    nc.vector.tensor_copy(zc, acc[HD:HD + 1, :])
                    rec = norms.tile([1, 512], f32, tag="rec", name="rec")
                    nc.vector.reciprocal_approx_fast(out=rec, in_=zc)
                    st["rec"] = rec
                def d2():
                    rb = fpp.tile([HD, 512], f32, tag="fp", name="rb")
                    nc.tensor.matmul(rb, lhsT=ones_f, rhs=st["rec"],
                                     start=True, stop=True)
                    rbs = norms.tile([HD, 512], f32, tag="rbs", name="rbs")
                    nc.vector.tensor_copy(rbs, rb)
                    st["rbs"] = rbs
                def d3():
                    if h == 0:
                        dst = stagingP[0:HD, q0]
                    elif h == 1:
                        dst = stagingP[HD:128, q0]
                    else:
                        dst = stagingC[0:HD, q0]
                    nc.vector.tensor_mul(dst, acc[0:HD, :], st["rbs"])
                tasks.append(d1)
                tasks.append(d2)
                tasks.append(d3)

            def push_outproj(bp):
                for nt in range(4 * bp, 4 * bp + 4):
                    nsl = slice(nt * 128, (nt + 1) * 128)
                    st = {}
                    def f1(nsl=nsl, st=st):
                        fs = fout.tile([128, C], f32, tag="fs", name="fs")
                        st["fs"] = fs
                        fp = fpp.tile([128, 512], f32, tag="fp", name="fp1")
                        nc.tensor.matmul(fp, lhsT=stagingP[:, nsl],
                                         rhs=wp_ab[:, 0:512],
                                         start=True, stop=False)
                        nc.tensor.matmul(fp, lhsT=stagingC[0:HD + 1, nsl],
                                         rhs=wp_c[0:HD + 1, 0:512],
                                         start=False, stop=True)
                        nc.vector.tensor_copy(fs[:, 0:512], fp)
                    def f2(nsl=nsl, st=st):
                        fp = fpp.tile([128, 256], f32, tag="fp", name="fp2")
                        nc.tensor.matmul(fp, lhsT=stagingP[:, nsl],
                                         rhs=wp_ab[:, 512:768],
                                         start=True, stop=False)
                        nc.tensor.matmul(fp, lhsT=stagingC[0:HD + 1, nsl],
                                         rhs=wp_c[0:HD + 1, 512:768],
                                         start=False, stop=True)
                        nc.vector.tensor_copy(st["fs"][:, 512:768], fp)
                        nc.sync.dma_start(out=T["out_d"][nsl, :], in_=st["fs"])
                    tasks.append(f1)
                    tasks.append(f2)

            pend_pv = deque()  # PV lags scores/exp by PVLAG chunks

            def run_chunk(h, bp, kp, acc, QT, KTt):
                q0 = slice(bp * 512, (bp + 1) * 512)
                kt0, kt1 = 2 * kp, 2 * kp + 1
                sc = scp.tile([128, 1024], f32, tag="sc", name="sc")
                nc.tensor.matmul(
                    sc[:, 0:512],
                    lhsT=KTt[0:HD, kt0 * 128:(kt0 + 1) * 128],
                    rhs=QT[0:HD, q0],
                    start=True, stop=True,
                )
                nc.tensor.matmul(
                    sc[:, 512:1024],
                    lhsT=KTt[HD:128, kt1 * 128:(kt1 + 1) * 128],
                    rhs=QT[HD:128, q0],
                    start=True, stop=True, tile_position=(64, 0),
                )
                ec = epool.tile([128, 1024], bf16, tag="ec", name="ec")
                g = gchunk[0]
                gchunk[0] += 1
                if not dve_chunk(g, kp):
                    nc.scalar.activation(ec, sc, EXP, bias=corr[:, 0:1])
                    if h == 0 and bp == 0 and kp == 0 and "ec_s" in taps:
                        nc.sync.dma_start(out=taps["ec_s"], in_=ec)
                else:
                    nc.vector.tensor_scalar(
                        ec.bitcast(i16), sc, ea[:, 0:1], eb[:, 0:1],
                        ALU.mult, ALU.add,
                    )
                    if h == 0 and bp == 0 and "ec_d" in taps and g == min(_KPSET) if KPAT.startswith("kp:") else False:
                        stt = tapp.tile([128, 1024], f32, tag="sct", name="stt")
                        nc.vector.tensor_copy(stt, sc)
                        nc.sync.dma_start(out=taps["sc_d"], in_=stt)
                        nc.sync.dma_start(out=taps["ec_d"], in_=ec)
                pend_pv.append(mk_pv(acc, h, ec, kp))

            for h in range(HPC):
                QT = QT_t[h]
                KTt = KT_t[h]
                nxt = h + 1 if h + 1 < HPC else None
                if h == 0:
                    # remaining head-0 qkproj blocks + V-proj interleaved;
                    # gated pops below guarantee writes are EMITTED before
                    # the chunk reads that depend on them
                    for blk in range(1, NBLK):
                        for stg in emit_qkproj_stages(0, blk):
                            qtasks.append(stg)
                        qtasks.append(lambda a=4 + (blk - 1) * 4: [
                            emit_vproj(nt) for nt in range(2 + a, min(NT, 6 + a))
                        ])
                    qtasks.append(lambda: emit_qk_dup(0))
                else:
                    # all tasks for this head must be emitted before its
                    # first chunk
                    need = (36, 102)[h - 1]
                    while qtasks and qpopped[0] < need:
                        qtasks.popleft()()
                        qpopped[0] += 1
                if nxt is not None:
                    if nxt + 1 < HPC:
                        emit_wqk_dma(nxt + 1)
                    for blk in range(NBLK):
                        for stg in emit_qkproj_stages(nxt, blk):
                            qtasks.append(stg)
                    qtasks.append(lambda nn=nxt: emit_qk_dup(nn))

                for bp in range(NBLK):
                    acc = accp.tile([HD + 1, 512], f32, tag="acc", name="acc")
                    for kp2 in range(0, NCH, 2):
                        if h == 0 and bp == 0:
                            # emission gate: qk block s+2 (and its vproj
                            # batch) must be emitted before these chunks
                            need = min(36, 5 * (kp2 // 2 + 3))
                            while qtasks and qpopped[0] < need:
                                qtasks.popleft()()
                                qpopped[0] += 1
                        if kp2 == 2 and deferred[0] is not None:
                            deferred[0]()
                            deferred[0] = None
                        run_chunk(h, bp, kp2, acc, QT, KTt)
                        run_chunk(h, bp, kp2 + 1, acc, QT, KTt)
                        while len(pend_pv) > PVLAG:
                            pend_pv.popleft()()
                        for _ in range(2 if tasks else 1):
                            if tasks:
                                tasks.popleft()()
                            elif qtasks:
                                qtasks.popleft()()
                                qpopped[0] += 1

                    def mkpush(acc=acc, h=h, bp=bp):
                        def p():
                            push_drain(acc, h, bp)
                            if h == HPC - 1:
                                push_outproj(bp)
                        return p
                    deferred[0] = mkpush()

            # flush
            while pend_pv:
                pend_pv.popleft()()
            if deferred[0] is not None:
                deferred[0]()
                deferred[0] = None
            while tasks:
                tasks.popleft()()
            while qtasks:
                qtasks.popleft()()

            if "sp" in taps:
                nc.sync.dma_start(out=taps["sp"], in_=stagingP)
                nc.sync.dma_start(out=taps["scg"], in_=stagingC)


def _get_nc():
    if "nc" not in _BUILT:
        _BUILT["nc"] = _build_nc()
    return _BUILT["nc"]


def kernel(x, w_qkv, b_qkv, w_proj, b_proj, _trace=None):
    from concourse import bass_utils

    in_maps = _host_inputs(x, w_qkv, b_qkv, w_proj, b_proj)
    nc = _get_nc()
    trace = bool(int(os.environ.get("TRACE_KERNEL", "0"))) if _trace is None else _trace
    res = bass_utils.run_bass_kernel_spmd(
        nc, in_maps, core_ids=list(range(NCORES)), trace=trace,
        trace_cores=list(range(NCORES)) if trace else None,
        stitch_traces=bool(trace),
    )
    _BUILT["last_results"] = res
    parts = [res.results[i]["out"] for i in range(NCORES)]
    out = np.empty((B, N, C), dtype=np.float32)
    for b in range(B):
        out[b] = parts[4 * b] + parts[4 * b + 1] + parts[4 * b + 2] + parts[4 * b + 3]
    return out
